# revision 1
# baseline (speedup 1.0000x reference)
"""Causal self-attention (B=4, T=2048, C=768, H=12) on 8 Trainium2 cores.

Sharding: core c handles batch b=c//2 and heads [6*(c%2), 6*(c%2)+6).
Each core computes its 6 heads end-to-end (qkv proj -> attention -> partial
c_proj); the host sums the two partial c_proj outputs per batch and adds
b_proj (tensor-parallel all-reduce done host-side).

Matmuls run in float32r (fp32 storage, reduced-precision multiplies at 4x
the fp32 PE rate); accumulation stays fp32 in PSUM.

Attention per head with S computed transposed (keys on partitions):
  S.T[k,q] = K.T @ Q per 128-key block; expS = exp(S.T * 1/sqrt(D)) fused on
  ScalarE (no max subtraction -- scores are bounded, exp stays in fp32 range);
  causal mask via affine_select on the diagonal block only;
  O'[65,q] += V'[kblock].T @ expS in PSUM, where V' carries an all-ones
  column so O'[64] accumulates the softmax denominators for free;
  O = O'[0:64] * recip(O'[64]) with a stream_shuffle partition broadcast.

Emission interleaves q/k projection with attention heads so ScalarE exp work
overlaps projection PE work (PSUM split: st 4 banks, O' 2, projections 2).
"""

import sys

sys.path.insert(0, "/opt/trn_rl_repo")

from contextlib import ExitStack

import numpy as np

import concourse.bass as bass
import concourse.tile as tile
from concourse import bacc, mybir, bass_utils

B, T, C, H = 4, 2048, 768, 12
D = C // H  # 64
HPC = H // 2  # heads per core = 6
NCORES = 8
QKC = 2 * HPC * D  # 768 q+k outcols per core
VC = HPC * (D + 1)  # 390 v cols (64 v + 1 ones per head)
KB = T // 128  # 16 key blocks
TB = T // 128  # 16 token blocks
CB = C // 128  # 6 contraction chunks
HT = T // 2  # 1024, q-half width

f32 = mybir.dt.float32
f32r = mybir.dt.float32r
ts = bass.ts
SCALE = 1.0 / float(np.sqrt(D))


def _emit(tc, xT, wqk, bqk, wv, wpc, y, dbg=None):
    nc = tc.nc
    Exp = mybir.ActivationFunctionType.Exp

    with ExitStack() as top:
        qkTp = top.enter_context(tc.tile_pool(name="qkTp", bufs=1))
        vtp = top.enter_context(tc.tile_pool(name="vtp", bufs=1))
        ocp = top.enter_context(tc.tile_pool(name="ocp", bufs=1))
        wp = top.enter_context(tc.tile_pool(name="wp", bufs=1))
        esp = top.enter_context(tc.tile_pool(name="esp", bufs=4))
        nrm = top.enter_context(tc.tile_pool(name="nrm", bufs=1))
        ohp = top.enter_context(tc.tile_pool(name="ohp", bufs=2))

        qkt = [qkTp.tile([128, T], f32r, tag=f"qkt{i}", name=f"qkt{i}") for i in range(CB)]
        vt = [vtp.tile([128, VC], f32r, tag=f"vt{t}", name=f"vt{t}") for t in range(TB)]
        ocat = [ocp.tile([128, T], f32r, tag=f"oc{i}", name=f"oc{i}") for i in range(3)]

        wqkt = [wp.tile([128, QKC], f32r, tag=f"wqk{i}", name=f"wqk{i}") for i in range(CB)]
        bqt = [wp.tile([128, 1], f32, tag=f"bq{i}", name=f"bq{i}") for i in range(CB)]

        def emit_qk_proj(ob, qkps):
            # qkT[128*ob : 128*(ob+1), :] = W@x.T + bias, token chunks of 512
            for tch in range(4):
                ps = qkps.tile([128, 512], f32, tag="qkps", name="qkps")
                for kc in range(CB):
                    nc.tensor.matmul(
                        ps[:],
                        wqkt[kc][:, ts(ob, 128)],
                        xt[kc][:, ts(tch, 512)],
                        start=(kc == 0),
                        stop=(kc == CB - 1),
                    )
                nc.vector.tensor_scalar_add(
                    qkt[ob][:, ts(tch, 512)], ps[:], bqt[ob][:, 0:1]
                )

        def norm_half(h, op, half):
            # O = O'[0:64] / O'[64] for q-half `half`; write into ocat
            bp = 64 * (h % 2)
            rb = nrm.tile([96, HT], f32, tag="rb", name="rb")
            nc.vector.tensor_copy(rb[D : D + 1, :], op[D : D + 1, :])
            nc.vector.stream_shuffle(rb[32:64, :], rb[64:96, :], mask=[0] * 32)
            nc.gpsimd.tensor_copy(rb[0:32, :], rb[32:64, :])
            rb2 = nrm.tile([64, HT], f32, tag="rb2", name="rb2")
            nc.vector.reciprocal_approx_fast(out=rb2[:], in_=rb[0:D, :])
            oh = ohp.tile([64, HT], f32r, tag="oh", name="oh")
            nc.vector.tensor_mul(oh[:], op[0:D, :], rb2[:])
            nc.sync.dma_start(
                ocat[h // 2][bp : bp + 64, half * HT : half * HT + HT], oh[:]
            )

        def emit_head(h, stp, opp):
            qt = qkt[h // 2]
            kt = qkt[3 + h // 2]
            bp = 64 * (h % 2)
            for half in range(2):
                hlo = half * HT
                hhi = hlo + HT
                op = opp.tile([D + 1, HT], f32, tag="op", name="op")
                for kb in range(min(KB, (hhi + 127) // 128)):
                    qs = 128 * kb
                    lo = max(hlo, qs)
                    if lo >= hhi:
                        continue
                    st = stp.tile([128, HT], f32, tag="st", name="st")
                    es = esp.tile([128, HT], f32r, tag="es", name="es")
                    # QK: S.T[k, q] in pieces of <=512 within one PSUM bank.
                    # f32r matmuls with free dim < 256 run at 1/4 rate, so
                    # when the leading piece would be 128 wide, widen it down
                    # to the 256 boundary; the extra columns land above the
                    # causal diagonal and are never read (exp starts at `lo`,
                    # PV pieces start at `qs`).
                    a = lo - 128 if lo % 512 == 384 else lo
                    while a < hhi:
                        b = min((a // 512 + 1) * 512, hhi)
                        nc.tensor.matmul(
                            st[:, a - hlo : b - hlo],
                            kt[bp : bp + 64, ts(kb, 128)],
                            qt[bp : bp + 64, a:b],
                            start=True,
                            stop=True,
                        )
                        a = b
                    nc.scalar.activation(
                        es[:, lo - hlo : HT], st[:, lo - hlo : HT], Exp, scale=SCALE
                    )
                    diag = hlo <= qs
                    if diag:
                        # causal mask on the diagonal block: keep q >= k
                        nc.gpsimd.affine_select(
                            out=es[:, qs - hlo : qs - hlo + 128],
                            in_=es[:, qs - hlo : qs - hlo + 128],
                            compare_op=mybir.AluOpType.is_ge,
                            fill=0.0,
                            base=0,
                            pattern=[[1, 128]],
                            channel_multiplier=-1,
                        )
                    # PV accumulate into O' (q-half slice); the diagonal
                    # 128-wide piece is split out so the rest doesn't wait
                    # on the mask
                    pieces = []
                    for qc in range(max(kb // 4, 2 * half), 2 * half + 2):
                        a = max(qs, 512 * qc)
                        b = 512 * (qc + 1)
                        if diag and a == qs and b > qs + 256:
                            # split so the non-diagonal part doesn't wait on
                            # the mask; start=True only on the first piece --
                            # it clears the whole bank's has_written bits
                            pieces.append((qs, qs + 256, kb == 0, False))
                            pieces.append((qs + 256, b, False, kb == 4 * qc + 3))
                        else:
                            pieces.append((a, b, kb == 0, kb == 4 * qc + 3))
                    for a, b, start_f, stop_f in pieces:
                        nc.tensor.matmul(
                            op[:, a - hlo : b - hlo],
                            vt[kb][:, 65 * h : 65 * h + 65],
                            es[:, a - hlo : b - hlo],
                            start=start_f,
                            stop=stop_f,
                        )
                norm_half(h, op, half)

        # ---------------- emission: projections interleaved with heads ----
        with ExitStack() as psA:
            stp = psA.enter_context(tc.tile_pool(name="stp", bufs=2, space="PSUM"))
            op1 = psA.enter_context(ExitStack())
            opp = op1.enter_context(tc.tile_pool(name="opp", bufs=1, space="PSUM"))

            with ExitStack() as xsc:
                xw = xsc.enter_context(tc.tile_pool(name="xw", bufs=1))
                xt = [
                    xw.tile([128, T], f32r, tag=f"xt{i}", name=f"xt{i}")
                    for i in range(CB)
                ]
                with ExitStack() as wvsc:
                    wvp = wvsc.enter_context(tc.tile_pool(name="wvp", bufs=1))
                    wvt = [
                        wvp.tile([128, VC], f32r, tag=f"wv{i}", name=f"wv{i}")
                        for i in range(CB)
                    ]
                    ones128 = wvp.tile([1, 128], f32r, tag="ones128", name="ones128")
                    nc.sync.dma_start(ones128[:], xT[C : C + 1, 0:128])
                    wvb = wvp.tile([1, VC], f32r, tag="wvb", name="wvb")
                    nc.sync.dma_start(wvb[:], wv[C : C + 1, :])
                    for i in range(CB):
                        nc.sync.dma_start(wvt[i][:], wv[ts(i, 128), :])
                        nc.sync.dma_start(bqt[i][:], bqk[ts(i, 128), :])
                    for tch in range(4):
                        for i in range(CB):
                            nc.sync.dma_start(
                                xt[i][:, ts(tch, 512)], xT[ts(i, 128), ts(tch, 512)]
                            )
                        if tch < 2:
                            for i in range(3 * tch, 3 * tch + 3):
                                nc.sync.dma_start(wqkt[i][:], wqk[ts(i, 128), :])

                    with tc.tile_pool(name="vps", bufs=2, space="PSUM") as vps:
                        for tb in range(TB):
                            ps = vps.tile([128, VC], f32, tag="vps", name="vps")
                            for kc in range(CB + 1):
                                if kc < CB:
                                    lhsT = xt[kc][:, ts(tb, 128)]
                                    rhs = wvt[kc][:]
                                else:
                                    lhsT = ones128[:, 0:128]
                                    rhs = wvb[:]
                                nc.tensor.matmul(
                                    ps[:], lhsT, rhs, start=(kc == 0), stop=(kc == CB)
                                )
                            nc.vector.tensor_copy(vt[tb][:], ps[:])

                with tc.tile_pool(name="qkps", bufs=2, space="PSUM") as qkps:
                    emit_qk_proj(0, qkps)
                    emit_qk_proj(3, qkps)
                    emit_head(0, stp, opp)
                    emit_qk_proj(1, qkps)
                    emit_qk_proj(4, qkps)
                    emit_head(1, stp, opp)
                    emit_qk_proj(2, qkps)
                    emit_qk_proj(5, qkps)

            op1.close()
            with tc.tile_pool(name="opp2", bufs=2, space="PSUM") as opp2:
                emit_head(2, stp, opp2)
                emit_head(3, stp, opp2)
                emit_head(4, stp, opp2)
                emit_head(5, stp, opp2)

        if dbg is not None:
            for i in range(CB):
                nc.sync.dma_start(dbg["qkT"][ts(i, 128), :], qkt[i][:].bitcast(f32))
            for t in range(TB):
                nc.sync.dma_start(dbg["v"][ts(t, 128), :], vt[t][:].bitcast(f32))
            for i in range(3):
                nc.sync.dma_start(dbg["oc"][ts(i, 128), :], ocat[i][:].bitcast(f32))

        # ---------------- output projection ----------------
        with ExitStack() as phC:
            wpp = phC.enter_context(tc.tile_pool(name="wpp", bufs=1))
            yop = phC.enter_context(tc.tile_pool(name="yop", bufs=3))
            yps = phC.enter_context(tc.tile_pool(name="yps", bufs=2, space="PSUM"))

            wpt = [wpp.tile([128, C], f32r, tag=f"wp{i}", name=f"wp{i}") for i in range(3)]
            for i in range(3):
                nc.sync.dma_start(wpt[i][:], wpc[ts(i, 128), :])

            for tb in range(TB):
                ps = yps.tile([128, C], f32, tag="yps", name="yps")
                for kc in range(3):
                    for a, w in ((0, 512), (512, 256)):
                        nc.tensor.matmul(
                            ps[:, a : a + w],
                            ocat[kc][:, ts(tb, 128)],
                            wpt[kc][:, a : a + w],
                            start=(kc == 0),
                            stop=(kc == 2),
                        )
                yt = yop.tile([128, C], f32, tag="yt", name="yt")
                if tb % 2 == 0:
                    nc.vector.tensor_copy(yt[:], ps[:])
                else:
                    nc.scalar.activation(
                        yt[:], ps[:], mybir.ActivationFunctionType.Copy
                    )
                nc.sync.dma_start(y[ts(tb, 128), :], yt[:])


_PROGRAM = None


def _build():
    global _PROGRAM
    if _PROGRAM is not None:
        return _PROGRAM
    nc = bacc.Bacc("TRN2", target_bir_lowering=False, debug=False, num_devices=NCORES)
    xT = nc.dram_tensor("xT", [C + 1, T], f32r, kind="ExternalInput").ap()
    wqk = nc.dram_tensor("wqk", [C, QKC], f32r, kind="ExternalInput").ap()
    bqk = nc.dram_tensor("bqk", [QKC, 1], f32, kind="ExternalInput").ap()
    wv = nc.dram_tensor("wv", [C + 1, VC], f32r, kind="ExternalInput").ap()
    wpc = nc.dram_tensor("wpc", [HPC * D, C], f32r, kind="ExternalInput").ap()
    y = nc.dram_tensor("y", [T, C], f32, kind="ExternalOutput").ap()
    with tile.TileContext(nc) as tc:
        _emit(tc, xT, wqk, bqk, wv, wpc, y)
    nc.compile()
    _PROGRAM = nc
    return nc


def _in_maps(x, w_qkv, b_qkv, w_proj):
    maps = []
    for c in range(NCORES):
        b = c // 2
        half = c % 2
        h0 = HPC * half  # first global head
        r0 = D * h0  # row offset within each of q/k/v sections
        span = HPC * D  # 384

        xTb = np.vstack([x[b].T, np.ones((1, T), np.float32)])  # [C+1, T]

        wq = w_qkv[r0 : r0 + span, :]
        wk = w_qkv[C + r0 : C + r0 + span, :]
        wqk = np.ascontiguousarray(np.vstack([wq, wk]).T)  # [C, 768]
        bqk = np.concatenate(
            [b_qkv[r0 : r0 + span], b_qkv[C + r0 : C + r0 + span]]
        ).reshape(QKC, 1)

        wv = np.zeros((C + 1, VC), dtype=np.float32)
        for hl in range(HPC):
            g = 2 * C + r0 + D * hl
            wv[0:C, 65 * hl : 65 * hl + D] = w_qkv[g : g + D, :].T
            wv[C, 65 * hl : 65 * hl + D] = b_qkv[g : g + D]
            wv[C, 65 * hl + D] = 1.0

        wpc = np.ascontiguousarray(w_proj[:, r0 : r0 + span].T)  # [384, C]

        maps.append(
            {
                "xT": xTb.astype(np.float32),
                "wqk": wqk.astype(np.float32),
                "bqk": bqk.astype(np.float32),
                "wv": wv,
                "wpc": wpc.astype(np.float32),
            }
        )
    return maps


def kernel(x, w_qkv, b_qkv, w_proj, b_proj, _trace=False):
    x = np.asarray(x, dtype=np.float32)
    w_qkv = np.asarray(w_qkv, dtype=np.float32)
    b_qkv = np.asarray(b_qkv, dtype=np.float32)
    w_proj = np.asarray(w_proj, dtype=np.float32)
    b_proj = np.asarray(b_proj, dtype=np.float32)

    nc = _build()
    maps = _in_maps(x, w_qkv, b_qkv, w_proj)
    res = bass_utils.run_bass_kernel_spmd(
        nc, maps, core_ids=list(range(NCORES)), trace=_trace
    )
    out = np.empty((B, T, C), dtype=np.float32)
    for b in range(B):
        out[b] = res.results[2 * b]["y"] + res.results[2 * b + 1]["y"] + b_proj
    if _trace:
        kernel._last_exec_time_ns = res.exec_time_ns
        kernel._last_results = res
    return out



# revision 2
# speedup vs baseline: 1.0170x; 1.0170x over previous
"""Causal self-attention (B=4, T=2048, C=768, H=12) on 8 Trainium2 cores.

Sharding: core c handles batch b=c//2 and heads [6*(c%2), 6*(c%2)+6).
Each core computes its 6 heads end-to-end (qkv proj -> attention -> partial
c_proj); the host sums the two partial c_proj outputs per batch and adds the
bias (v-bias is folded into the host-side bias since softmax weights sum to 1).

All matmul operands are fp16 (1 cycle/row on PE at any width, half the DMA
of f32); accumulation stays fp32 in PSUM.

Attention per head:
  S.T[k,q] = K.T @ Q per 128-key block (keys on partitions);
  es = exp(S.T * 1/sqrt(D)) fused on ScalarE, fp16 out;
  causal mask via affine_select on the diagonal 128x128 block only;
  PV transposed: O[q,d] = es(kb-block).T @ V[kb] accumulated over kb in PSUM,
  65-wide rhs per head (64 v-dims + ones column -> softmax denominators land
  per-partition);
  normalization = strided reciprocal + per-head tensor_scalar_mul (denom is a
  per-partition scalar in this layout - no broadcast machinery needed);
  O[q,d] pairs of heads are flipped to [d,q] for c_proj via DMA-engine
  transposes (idle resource; no PE/DVE cost).

Emission interleaves projections and attention phases so ScalarE exp work
(the co-bottleneck, ~110us) overlaps PE work (~130us) throughout: q/k blocks
are produced in descending-kb order so exp starts ~3us into the kernel.
"""

import sys

sys.path.insert(0, "/opt/trn_rl_repo")

from contextlib import ExitStack

import numpy as np

import concourse.bass as bass
import concourse.tile as tile
from concourse import bacc, mybir, bass_utils

B, T, C, H = 4, 2048, 768, 12
D = C // H  # 64
HPC = H // 2  # heads per core = 6
NCORES = 8
CB = C // 128  # 6 contraction chunks
KB = T // 128  # 16 key blocks
TB = T // 128  # 16 token blocks
VC = HPC * (D + 1)  # 390 v cols incl ones
SPAN = HPC * D  # 384

f32 = mybir.dt.float32
f16 = mybir.dt.float16
ts = bass.ts
SCALE = 1.0 / float(np.sqrt(D))
Exp = mybir.ActivationFunctionType.Exp
Copy = mybir.ActivationFunctionType.Copy


def _emit(tc, xT, wqk, bqk, wv, wpc, y, dbg=None):
    nc = tc.nc

    with ExitStack() as top:
        xw = top.enter_context(tc.tile_pool(name="xw", bufs=1))
        wp = top.enter_context(tc.tile_pool(name="wp", bufs=1))
        qkp = top.enter_context(tc.tile_pool(name="qkp", bufs=1))
        vtp = top.enter_context(tc.tile_pool(name="vtp", bufs=1))
        esp = top.enter_context(tc.tile_pool(name="esp", bufs=2))
        osb = top.enter_context(tc.tile_pool(name="osb", bufs=2))
        rp = top.enter_context(tc.tile_pool(name="rp", bufs=4))
        ocp = top.enter_context(tc.tile_pool(name="ocp", bufs=1))
        yop = top.enter_context(tc.tile_pool(name="yop", bufs=3))

        pjp = top.enter_context(tc.tile_pool(name="pjp", bufs=2, space="PSUM"))
        stp = top.enter_context(tc.tile_pool(name="stp", bufs=2, space="PSUM"))
        opp = top.enter_context(tc.tile_pool(name="opp", bufs=2, space="PSUM"))

        xt = [xw.tile([128, T], f16, tag=f"xt{i}", name=f"xt{i}") for i in range(CB)]
        wqkt = [wp.tile([128, C], f16, tag=f"wqk{i}", name=f"wqk{i}") for i in range(CB)]
        bqt = [wp.tile([128, 1], f32, tag=f"bq{i}", name=f"bq{i}") for i in range(CB)]
        wvt = [wp.tile([128, SPAN], f16, tag=f"wv{i}", name=f"wv{i}") for i in range(CB)]
        qkt = [qkp.tile([128, T], f16, tag=f"qkt{i}", name=f"qkt{i}") for i in range(CB)]
        vt = [vtp.tile([128, VC], f16, tag=f"vt{t}", name=f"vt{t}") for t in range(TB)]
        ocat = [ocp.tile([128, T], f16, tag=f"oc{i}", name=f"oc{i}") for i in range(3)]
        wpt = [wp.tile([128, C], f16, tag=f"wp{i}", name=f"wp{i}") for i in range(3)]

        # ---------------- DMA loads ----------------
        # tch3 slices first so descending-kb S for head 0 can start early.
        for i in (0, 3):
            nc.sync.dma_start(wqkt[i][:], wqk[ts(i, 128), :])
            nc.sync.dma_start(bqt[i][:], bqk[ts(i, 128), :])
        for i in range(CB):
            nc.sync.dma_start(xt[i][:, 1536:2048], xT[ts(i, 128), 1536:2048])
        for i in range(CB):
            nc.sync.dma_start(wvt[i][:], wv[ts(i, 128), :])
        for i in range(CB):
            nc.sync.dma_start(xt[i][:, 0:1536], xT[ts(i, 128), 0:1536])
        for i in (1, 4, 2, 5):
            nc.sync.dma_start(wqkt[i][:], wqk[ts(i, 128), :])
            nc.sync.dma_start(bqt[i][:], bqk[ts(i, 128), :])
        for i in range(3):
            nc.sync.dma_start(wpt[i][:], wpc[ts(i, 128), :])

        # ---------------- emitters ----------------
        def emit_qk_proj(ob, tch):
            ps = pjp.tile([128, 512], f32, tag="pj", name="pj")
            for kc in range(CB):
                nc.tensor.matmul(
                    ps[:],
                    wqkt[kc][:, ts(ob, 128)],
                    xt[kc][:, ts(tch, 512)],
                    start=(kc == 0),
                    stop=(kc == CB - 1),
                )
            nc.vector.tensor_scalar_add(
                qkt[ob][:, ts(tch, 512)], ps[:], bqt[ob][:, 0:1]
            )

        def emit_v_proj(tb):
            ps = pjp.tile([128, SPAN], f32, tag="pj", name="pjv")
            for kc in range(CB):
                nc.tensor.matmul(
                    ps[:],
                    xt[kc][:, ts(tb, 128)],
                    wvt[kc][:],
                    start=(kc == 0),
                    stop=(kc == CB - 1),
                )
            # strided copy into 65-col head slots; ones col via memset
            nc.vector.tensor_copy(
                vt[tb][:].rearrange("p (h d) -> p h d", h=HPC, d=65)[:, :, 0:D],
                ps[:].rearrange("p (h d) -> p h d", h=HPC, d=D),
            )
            nc.gpsimd.memset(vt[tb][:, D:VC:65], 1.0)

        def emit_s_exp(h, kb, es):
            # S.T [128 keys, q] for q in [128*kb, 2048); es[kb] = exp(scale*S.T)
            bp = D * (h % 2)
            qt = qkt[h // 2]
            kt = qkt[3 + h // 2][bp : bp + D, ts(kb, 128)]
            base = 128 * kb
            W = T - base
            e = esp.tile([128, W], f16, tag=f"es{kb}", name=f"es{kb}_{h}")
            es[kb] = e
            for p0 in range(base, T, 1024):
                pw = min(1024, T - p0)
                st = stp.tile([128, pw], f32, tag="st", name="st")
                for sub in range(p0, p0 + pw, 512):
                    sw = min(512, p0 + pw - sub)
                    nc.tensor.matmul(
                        st[:, sub - p0 : sub - p0 + sw],
                        kt,
                        qt[bp : bp + D, sub : sub + sw],
                        start=True,
                        stop=True,
                    )
                nc.scalar.activation(
                    e[:, p0 - base : p0 - base + pw], st[:], Exp, scale=SCALE
                )
            # causal mask on the diagonal block: keep q >= k
            nc.gpsimd.affine_select(
                out=e[:, 0:128],
                in_=e[:, 0:128],
                compare_op=mybir.AluOpType.is_ge,
                fill=0.0,
                base=0,
                pattern=[[1, 128]],
                channel_multiplier=-1,
            )

        def emit_pv_group(h, g, es, o2s):
            # qb in [4g, 4g+4): O[q, 0:64] + denom col, all in one PSUM bank
            op = opp.tile([128, 260], f32, tag="op", name="op")
            first = True
            for j in range(4):
                qb = 4 * g + j
                c0 = 65 * j
                for kb in range(qb + 1):
                    nc.tensor.matmul(
                        op[:, c0 : c0 + 65],
                        es[kb][:, 128 * (qb - kb) : 128 * (qb - kb) + 128],
                        vt[kb][:, 65 * h : 65 * h + 65],
                        start=first,
                        stop=(kb == qb),
                    )
                    first = False
            r = rp.tile([128, 4], f32, tag="r", name="r")
            nc.vector.reciprocal(r[:], op[:, D : 260 : 65])
            bp = D * (h % 2)
            for j in range(4):
                qb = 4 * g + j
                nc.vector.tensor_scalar_mul(
                    o2s[qb][:, bp : bp + D], op[:, 65 * j : 65 * j + D], r[:, j : j + 1]
                )
                if h % 2 == 1:
                    nc.sync.dma_start_transpose(
                        ocat[h // 2][:, ts(qb, 128)], o2s[qb][:]
                    )

        def emit_cproj(tb):
            ps = stp.tile([128, C], f32, tag="st", name="yps")
            for kc in range(3):
                for a, w in ((0, 512), (512, 256)):
                    nc.tensor.matmul(
                        ps[:, a : a + w],
                        ocat[kc][:, ts(tb, 128)],
                        wpt[kc][:, a : a + w],
                        start=(kc == 0),
                        stop=(kc == 2),
                    )
            yt = yop.tile([128, C], f32, tag="yt", name="yt")
            nc.scalar.activation(yt[:], ps[:], Copy)
            nc.sync.dma_start(y[ts(tb, 128), :], yt[:])

        # ---------------- schedule ----------------
        es_cur = {}   # es tiles of head being s/exp'd
        es_prev = {}  # es tiles of head being pv'd

        def phase1(h, fillers):
            # descending kb; fillers: list of thunks interleaved (one per kb)
            fi = iter(fillers)
            for kb in range(KB - 1, -1, -1):
                if kb % 4 == 3:
                    for f in next(fi, ()):
                        f()
                emit_s_exp(h, kb, es_cur)
                for f in next(fi, ()):
                    f()

        # prologue + head 0 phase1, v-proj as filler
        fillers0 = [
            [lambda: emit_qk_proj(0, 3), lambda: emit_qk_proj(3, 3)],
            [lambda tb=15: emit_v_proj(tb)],
            [lambda tb=14: emit_v_proj(tb)],
            [lambda tb=13: emit_v_proj(tb)],
            [lambda: emit_qk_proj(0, 2), lambda: emit_qk_proj(3, 2)],
            [lambda tb=12: emit_v_proj(tb)],
            [lambda tb=11: emit_v_proj(tb)],
            [lambda tb=10: emit_v_proj(tb)],
            [lambda: emit_qk_proj(0, 1), lambda: emit_qk_proj(3, 1)],
            [lambda tb=9: emit_v_proj(tb)],
            [lambda tb=8: emit_v_proj(tb)],
            [lambda tb=7: emit_v_proj(tb)],
            [lambda: emit_qk_proj(0, 0), lambda: emit_qk_proj(3, 0)],
            [lambda tb=6: emit_v_proj(tb)],
            [lambda tb=5: emit_v_proj(tb)],
            [lambda tb=4: emit_v_proj(tb)],
            [lambda tb=3: emit_v_proj(tb)],
            [lambda tb=2: emit_v_proj(tb)],
            [lambda tb=1: emit_v_proj(tb)],
            [lambda tb=0: emit_v_proj(tb)],
        ]
        phase1(0, fillers0)

        # per-slot qk-proj fillers for heads 2..5's chunks (desc tch to match
        # descending-kb consumption of the NEXT slot's phase1)
        slot_projs = {
            1: [(1, 3), (4, 3), (1, 2), (4, 2), (1, 1), (4, 1), (1, 0), (4, 0)],
            2: [(2, 3), (5, 3), (2, 2), (5, 2)],
            3: [(2, 1), (5, 1), (2, 0), (5, 0)],
        }

        o2s = None
        for h in range(HPC):
            # start of a head pair: fresh O2 tiles [128 q, 128 d(2 heads)]
            if h % 2 == 0:
                o2s = [
                    osb.tile([128, 128], f16, tag=f"o2q{qb}", name=f"o2q{qb}_{h}")
                    for qb in range(TB)
                ]
            es_prev, es_cur = es_cur, {}
            projs = slot_projs.get(h + 1, [])
            nexth = h + 1 if h + 1 < HPC else None

            # interleave: pv groups of head h, s/exp of head h+1, projs
            pj = iter(projs)

            def fill(n):
                out = []
                for _ in range(n):
                    p = next(pj, None)
                    if p is not None:
                        out.append(lambda p=p: emit_qk_proj(*p))
                return out

            if nexth is not None:
                fillers = [
                    [lambda: emit_pv_group(h, 0, es_prev, o2s)] + fill(1),
                    fill(1),
                    [lambda: emit_pv_group(h, 1, es_prev, o2s)] + fill(1),
                    fill(1),
                    [lambda: emit_pv_group(h, 2, es_prev, o2s)] + fill(1),
                    fill(1),
                    [lambda: emit_pv_group(h, 3, es_prev, o2s)] + fill(1),
                    fill(1),
                ]
                phase1(nexth, fillers)
            else:
                # tail: last head's pv + cproj interleaved
                for g in range(4):
                    emit_pv_group(h, g, es_prev, o2s)
                    for tb in range(4 * g, 4 * g + 4):
                        emit_cproj(tb)

        if dbg is not None:
            for i in range(CB):
                nc.sync.dma_start(dbg["qkT"][ts(i, 128), :], qkt[i][:])
            for t in range(TB):
                nc.sync.dma_start(dbg["v"][ts(t, 128), :], vt[t][:])
            for i in range(3):
                nc.sync.dma_start(dbg["oc"][ts(i, 128), :], ocat[i][:])


_PROGRAM = None


def _build(dbg=False):
    global _PROGRAM
    if _PROGRAM is not None and not dbg:
        return _PROGRAM
    nc = bacc.Bacc("TRN2", target_bir_lowering=False, debug=False, num_devices=NCORES)
    xT = nc.dram_tensor("xT", [C, T], f16, kind="ExternalInput").ap()
    wqk = nc.dram_tensor("wqk", [C, C], f16, kind="ExternalInput").ap()
    bqk = nc.dram_tensor("bqk", [C, 1], f32, kind="ExternalInput").ap()
    wv = nc.dram_tensor("wv", [C, SPAN], f16, kind="ExternalInput").ap()
    wpc = nc.dram_tensor("wpc", [SPAN, C], f16, kind="ExternalInput").ap()
    y = nc.dram_tensor("y", [T, C], f32, kind="ExternalOutput").ap()
    dbgd = None
    if dbg:
        dbgd = {
            "qkT": nc.dram_tensor("dbg_qkT", [C, T], f16, kind="ExternalOutput").ap(),
            "v": nc.dram_tensor("dbg_v", [T, VC], f16, kind="ExternalOutput").ap(),
            "oc": nc.dram_tensor("dbg_oc", [SPAN, T], f16, kind="ExternalOutput").ap(),
        }
    with tile.TileContext(nc) as tc:
        _emit(tc, xT, wqk, bqk, wv, wpc, y, dbg=dbgd)
    nc.compile()
    if not dbg:
        _PROGRAM = nc
    return nc


def _in_maps(x, w_qkv, b_qkv, w_proj):
    maps = []
    for c in range(NCORES):
        b = c // 2
        half = c % 2
        r0 = SPAN * half

        wq = w_qkv[r0 : r0 + SPAN, :]
        wk = w_qkv[C + r0 : C + r0 + SPAN, :]
        wqk = np.ascontiguousarray(np.vstack([wq, wk]).T)  # [C, 768]
        bqk = np.concatenate(
            [b_qkv[r0 : r0 + SPAN], b_qkv[C + r0 : C + r0 + SPAN]]
        ).reshape(C, 1)
        wv = np.ascontiguousarray(w_qkv[2 * C + r0 : 2 * C + r0 + SPAN, :].T)
        wpc = np.ascontiguousarray(w_proj[:, r0 : r0 + SPAN].T)  # [384, C]

        maps.append(
            {
                "xT": np.ascontiguousarray(x[b].T).astype(np.float16),
                "wqk": wqk.astype(np.float16),
                "bqk": bqk.astype(np.float32),
                "wv": wv.astype(np.float16),
                "wpc": wpc.astype(np.float16),
            }
        )
    return maps


def kernel(x, w_qkv, b_qkv, w_proj, b_proj, _trace=False):
    x = np.asarray(x, dtype=np.float32)
    w_qkv = np.asarray(w_qkv, dtype=np.float32)
    b_qkv = np.asarray(b_qkv, dtype=np.float32)
    w_proj = np.asarray(w_proj, dtype=np.float32)
    b_proj = np.asarray(b_proj, dtype=np.float32)

    nc = _build()
    maps = _in_maps(x, w_qkv, b_qkv, w_proj)
    res = bass_utils.run_bass_kernel_spmd(
        nc, maps, core_ids=list(range(NCORES)), trace=_trace
    )
    # v-bias contributes sum_k a_k * bv = bv per token; fold through c_proj.
    b_eff = b_proj + w_proj @ b_qkv[2 * C : 3 * C]
    out = np.empty((B, T, C), dtype=np.float32)
    for b in range(B):
        out[b] = res.results[2 * b]["y"] + res.results[2 * b + 1]["y"] + b_eff
    if _trace:
        kernel._last_exec_time_ns = res.exec_time_ns
        kernel._last_results = res
    return out
